# revision 13
# baseline (speedup 1.0000x reference)
"""Trainium2 Bass kernel for spatial-reduction attention (PVT-style).

Per-core sharding (8 cores): core c handles batch b = c//2, head-group
g = c%2 (4 of 8 heads). Within a core everything is fused:
  - q/k/v projections (depthwise-conv spatial reduction + BN folded into
    per-tap projection weights on the host)
  - scores computed transposed: s.T[nk, nq] = k @ q.T (+rel.T), exp on ACT
  - o.T = [v | 1].T @ e.T gives attention output and softmax denominator
    in one accumulation; per-head normalization via partition-broadcast
  - final projection from the stacked normalized heads
Host sums the two head-group partial outputs per batch and adds bp.
"""

import os
import sys

import numpy as np

for _p in ("/opt/trn_rl_repo",):
    if _p not in sys.path and os.path.isdir(_p):
        sys.path.insert(0, _p)

import concourse.bacc as bacc
import concourse.tile as tile
from concourse import library_config, mybir
from concourse.alu_op_type import AluOpType
from concourse.bass_utils import run_bass_kernel_spmd

# Problem shapes (hardcoded per contract)
B = 4
N = 4096          # H*W = 64*64
C = 384
NK = 1024         # (H/2)*(W/2)
HD = 48           # head dim
NHL = 4           # heads per core (8 heads / 2 groups)
GC = 192          # channels per group = NHL*HD
SCALE = HD ** -0.5
BN_EPS = 1e-5

F32 = mybir.dt.float32
F32R = mybir.dt.float32r
BF16 = mybir.dt.bfloat16

# Precision knobs
USE_F32R = True          # use tf32-rate matmuls
REL_BF16 = os.environ.get("TRNK_REL_BF16", "1") == "1"   # rel streamed as bf16

_CACHE = {}


DT_MM = F32R if USE_F32R else F32


def _mm_dt(ap):
    return ap


def build_program():
    """Build the SPMD Bass program (same on every core)."""
    nc = bacc.Bacc("TRN2", target_bir_lowering=False, debug=False, num_devices=8)

    rel_dt = BF16 if REL_BF16 else F32

    xT = nc.dram_tensor("xT", [C, N], DT_MM, kind="ExternalInput").ap()
    wq = nc.dram_tensor("wq", [C, GC], DT_MM, kind="ExternalInput").ap()
    wk = nc.dram_tensor("wk", [4, C, GC], DT_MM, kind="ExternalInput").ap()
    kc = nc.dram_tensor("kc", [HD, NHL], F32, kind="ExternalInput").ap()
    wv = nc.dram_tensor("wv", [4, C, 272], DT_MM, kind="ExternalInput").ap()
    vc = nc.dram_tensor("vc", [1, 272], DT_MM, kind="ExternalInput").ap()
    wp = nc.dram_tensor("wp", [2, 128, C], DT_MM, kind="ExternalInput").ap()
    on1 = nc.dram_tensor("on1", [1, 128], DT_MM, kind="ExternalInput").ap()
    zz = nc.dram_tensor("zz", [16, NK], DT_MM, kind="ExternalInput").ap()
    relT = nc.dram_tensor("relT", [NHL, NK, N], rel_dt, kind="ExternalInput").ap()
    out = nc.dram_tensor("out", [N, C], F32, kind="ExternalOutput").ap()

    with tile.TileContext(nc) as tc:
        _build_kernel(tc, xT, wq, wk, kc, wv, vc, wp, on1, zz, relT, out)

    nc.compile()
    return nc


def _build_kernel(tc, xT, wq, wk, kc, wv, vc, wp, on1, zz, relT, out):
    nc = tc.nc
    Exp = mybir.ActivationFunctionType.Exp
    Ident = mybir.ActivationFunctionType.Identity

    with (
        tc.tile_pool(name="singles", bufs=1) as singles,
        tc.tile_pool(name="persist", bufs=1) as persist,
        tc.tile_pool(name="mpsum", bufs=2, space="PSUM") as mpsum,
        tc.tile_pool(name="spsum", bufs=3, space="PSUM") as spsum,
        tc.tile_pool(name="opsum", bufs=3, space="PSUM") as opsum,
        tc.tile_pool(name="epool", bufs=3) as epool,
        tc.tile_pool(name="relpool", bufs=3) as relpool,
        tc.tile_pool(name="stackp", bufs=2) as stackp,
        tc.tile_pool(name="outp", bufs=2) as outp,
        tc.tile_pool(name="smallp", bufs=2) as smallp,
    ):
        # gpsimd ucode library with PartitionBroadcast
        nc.gpsimd.load_library(library_config.attn)

        # ---- constant / weight loads ----
        # xs holds x.T permuted tap-major: [p, cc, tap, nk]; the whole
        # attention pipeline runs in this permuted nq order and the output
        # DMA un-permutes.
        xs_sb = singles.tile([128, 3, 4, NK], DT_MM, tag="xt")
        nc.sync.dma_start(out=xs_sb, in_=xT.rearrange("(cc p) (t k) -> p cc t k", p=128, t=4))

        wq_sb = singles.tile([128, 3, GC], DT_MM, tag="wq")
        nc.sync.dma_start(out=wq_sb, in_=wq.rearrange("(cc p) d -> p cc d", p=128))

        wk_sb = singles.tile([128, 4, 3, GC], DT_MM, tag="wk")
        nc.sync.dma_start(out=wk_sb, in_=wk.rearrange("t (cc p) d -> p t cc d", p=128))

        wv_sb = singles.tile([128, 4, 3, 272], DT_MM, tag="wv")
        nc.sync.dma_start(out=wv_sb, in_=wv.rearrange("t (cc p) d -> p t cc d", p=128))

        kc_sb = singles.tile([HD, NHL], F32, tag="kc")
        nc.sync.dma_start(out=kc_sb, in_=kc)

        vc_sb = singles.tile([1, 272], DT_MM, tag="vc")
        nc.sync.dma_start(out=vc_sb, in_=vc)

        wp_sb = singles.tile([128, 2, C], DT_MM, tag="wp")
        nc.sync.dma_start(out=wp_sb, in_=wp.rearrange("c p d -> p c d"))

        ones1 = singles.tile([1, 128], DT_MM, tag="ones1")
        nc.sync.dma_start(out=ones1, in_=on1)

        # ---- stage A: projections ----
        qT_sb = [persist.tile([HD, N], BF16, tag=f"qT{h}", name=f"qT{h}") for h in range(NHL)]
        kT_sb = [persist.tile([HD, NK], BF16, tag=f"kT{h}", name=f"kT{h}") for h in range(NHL)]
        v_sb = [persist.tile([128, 272], DT_MM, tag=f"v{c8}", name=f"v{c8}") for c8 in range(8)]

        # q.T per head: [48, 512] psum tiles, K=3x128 accumulation
        for h in range(NHL):
            hs = slice(h * HD, (h + 1) * HD)
            for t8 in range(8):
                ts = slice(t8 * 512, (t8 + 1) * 512)
                pq = mpsum.tile([HD, 512], F32, tag="mp")
                for cc in range(3):
                    nc.tensor.matmul(
                        pq,
                        lhsT=_mm_dt(wq_sb[:, cc, hs]),
                        rhs=_mm_dt(xs_sb[:, cc, :, :].rearrange("p t k -> p (t k)")[:, ts]),
                        start=(cc == 0),
                        stop=(cc == 2),
                    )
                nc.vector.tensor_copy(qT_sb[h][:, ts], pq)

        # k.T per head with conv folded: 4 taps x 3 c-chunks, + BN-const bias
        for h in range(NHL):
            hs = slice(h * HD, (h + 1) * HD)
            for t2 in range(2):
                ts = slice(t2 * 512, (t2 + 1) * 512)
                pk = mpsum.tile([HD, 512], F32, tag="mp")
                first = True
                for tap in range(4):
                    for cc in range(3):
                        nc.tensor.matmul(
                            pk,
                            lhsT=_mm_dt(wk_sb[:, tap, cc, hs]),
                            rhs=_mm_dt(xs_sb[:, cc, tap, ts]),
                            start=first,
                            stop=(tap == 3 and cc == 2),
                        )
                        first = False
                nc.scalar.activation(
                    kT_sb[h][:, ts], pk, Ident, bias=kc_sb[:, h : h + 1]
                )

        # v per nk-chunk: [128, 256] (4 heads x 49 cols + pad); ones column
        # and BN-const come from a K=1 matmul against vc.
        for c8 in range(8):
            pv = mpsum.tile([128, 272], F32, tag="mp")
            first = True
            for tap in range(4):
                for cc in range(3):
                    nc.tensor.matmul(
                        pv,
                        lhsT=_mm_dt(xs_sb[:, cc, tap, c8 * 128 : (c8 + 1) * 128]),
                        rhs=_mm_dt(wv_sb[:, tap, cc, :]),
                        start=first,
                        stop=False,
                    )
                    first = False
            nc.tensor.matmul(
                pv, lhsT=_mm_dt(ones1), rhs=_mm_dt(vc_sb), start=False, stop=True
            )
            nc.vector.tensor_copy(v_sb[c8], pv)

        # ---- stage B: attention ----
        for q in range(4):  # nq quarters of 1024
            SP = [stackp.tile([128, 1024], DT_MM, tag=f"sp{i}", name=f"sp{i}") for i in range(2)]
            for i in range(2):
                nc.sync.dma_start(out=SP[i][48:64, :], in_=zz)
                nc.sync.dma_start(out=SP[i][112:128, :], in_=zz)
            for h in range(NHL):
                eIn = [
                    epool.tile([128, 4, 1024], DT_MM, tag="eIn", name="eIn") for _ in range(2)
                ]
                po = [opsum.tile([65, 512], F32, tag="op", name="op") for _ in range(2)]
                for c in range(8):
                    rt = relpool.tile(
                        [128, 1024], BF16 if REL_BF16 else F32, tag="rel"
                    )
                    nc.sync.dma_start(
                        out=rt,
                        in_=relT[
                            h, c * 128 : (c + 1) * 128, q * 1024 : (q + 1) * 1024
                        ],
                    )
                    eI = eIn[c // 4]
                    ci = c % 4
                    for t2 in range(2):
                        ps = spsum.tile([128, 512], F32, tag="sp")
                        nc.tensor.matmul(
                            ps,
                            lhsT=kT_sb[h][:, c * 128 : (c + 1) * 128],
                            rhs=qT_sb[h][
                                :, q * 1024 + t2 * 512 : q * 1024 + (t2 + 1) * 512
                            ],
                            start=True,
                            stop=True,
                        )
                        nc.vector.tensor_tensor(
                            eI[:, ci, t2 * 512 : (t2 + 1) * 512],
                            ps,
                            rt[:, t2 * 512 : (t2 + 1) * 512],
                            AluOpType.add,
                        )
                for half in range(2):
                    flat = eIn[half].rearrange("p a b -> p (a b)")
                    nc.scalar.activation(flat, flat, Exp)
                for c in range(8):
                    eI = eIn[c // 4]
                    ci = c % 4
                    for t2 in range(2):
                        nc.tensor.matmul(
                            po[t2],
                            lhsT=_mm_dt(v_sb[c][:, h * 68 : h * 68 + 65]),
                            rhs=_mm_dt(eI[:, ci, t2 * 512 : (t2 + 1) * 512]),
                            start=(c == 0),
                            stop=(c == 7),
                        )
                for t2 in range(2):
                    rc = smallp.tile([1, 512], F32, tag="rc")
                    nc.vector.reciprocal(rc, po[t2][64:65, :])
                    rb = smallp.tile([HD, 512], F32, tag="rb")
                    nc.gpsimd.partition_broadcast(rb, rc)
                    nc.vector.tensor_tensor(
                        SP[h // 2][
                            (h % 2) * 64 : (h % 2) * 64 + HD,
                            t2 * 512 : (t2 + 1) * 512,
                        ],
                        po[t2][0:HD, :],
                        rb,
                        AluOpType.mult,
                    )
            # final projection for this quarter
            for sub in range(8):
                ss = slice(sub * 128, (sub + 1) * 128)
                pr = mpsum.tile([128, C], F32, tag="mp")
                nc.tensor.matmul(
                    pr,
                    lhsT=_mm_dt(SP[0][:, ss]),
                    rhs=_mm_dt(wp_sb[:, 0, :]),
                    start=True,
                    stop=False,
                )
                nc.tensor.matmul(
                    pr,
                    lhsT=_mm_dt(SP[1][:, ss]),
                    rhs=_mm_dt(wp_sb[:, 1, :]),
                    start=False,
                    stop=True,
                )
                ro = outp.tile([128, C], F32, tag="ro")
                nc.vector.tensor_copy(ro, pr)
                ov = out.rearrange(
                    "(i a j b) c -> a b i j c", i=32, a=2, j=32, b=2
                )
                nc.sync.dma_start(
                    out=ov[q // 2, q % 2, sub * 4 : sub * 4 + 4, :, :],
                    in_=ro,
                )


def _prep_core_inputs(x, relative_pos, Wq, Wk, Wv, sr_w, sr_b,
                      bn_gamma, bn_beta, bn_mean, bn_var):
    """Host-side input prep: one dict per core."""
    inv = bn_gamma / np.sqrt(bn_var + BN_EPS)           # [C]
    w_eff = sr_w[:, 0, :, :].reshape(C, 4) * inv[:, None]   # [C, 4] taps (a,b)
    beta_eff = (sr_b - bn_mean) * inv + bn_beta         # [C]

    rel_np = np.ascontiguousarray(relative_pos)
    if REL_BF16:
        import ml_dtypes
        rel_np = rel_np.astype(ml_dtypes.bfloat16)

    in_maps = []
    for core in range(8):
        b, g = core // 2, core % 2
        ds = slice(g * GC, (g + 1) * GC)

        wq_h = np.ascontiguousarray((Wq[ds, :] * SCALE).T)          # [C, GC]

        wk_h = np.empty((4, C, GC), np.float32)
        for tap in range(4):
            wk_h[tap] = Wk[ds, :].T * w_eff[:, tap][:, None]
        kc_full = Wk[ds, :] @ beta_eff                               # [GC]
        kc_h = np.ascontiguousarray(kc_full.reshape(NHL, HD).T)      # [HD, NHL]

        wv_h = np.zeros((4, C, 272), np.float32)
        vc_h = np.zeros((1, 272), np.float32)
        vconst = Wv[ds, :] @ beta_eff                                # [GC]
        for hl in range(NHL):
            cs = slice(hl * 68, hl * 68 + HD)
            rows = slice(hl * HD, (hl + 1) * HD)
            for tap in range(4):
                wv_h[tap, :, cs] = (Wv[ds, :][rows].T
                                    * w_eff[:, tap][:, None])
            vc_h[0, cs] = vconst[rows]
            vc_h[0, hl * 68 + 64] = 1.0

        wpT = Wp_slice_T(g)                                  # [GC, C]
        wp_h = np.zeros((2, 128, C), np.float32)
        for ch in range(2):
            wp_h[ch, 0:HD] = wpT[(2 * ch) * HD:(2 * ch + 1) * HD]
            wp_h[ch, 64:64 + HD] = wpT[(2 * ch + 1) * HD:(2 * ch + 2) * HD]

        # rel.T with permuted nq axis to match the device's nq ordering
        rg = rel_np[g * NHL : (g + 1) * NHL].transpose(0, 2, 1)  # [NHL, NK, N]
        relT_h = np.ascontiguousarray(
            rg.reshape(NHL, NK, 32, 2, 32, 2)
              .transpose(0, 1, 3, 5, 2, 4)
              .reshape(NHL, NK, N)
        )

        # x.T permuted tap-major: [C, 4, NK] flattened to [C, N]
        xp = np.ascontiguousarray(
            x[b].reshape(32, 2, 32, 2, C).transpose(4, 1, 3, 0, 2).reshape(C, N)
        )
        in_maps.append({
            "on1": np.ones((1, 128), np.float32),
            "zz": np.zeros((16, NK), np.float32),
            "xT": xp.astype(np.float32),
            "wq": wq_h.astype(np.float32),
            "wk": wk_h.astype(np.float32),
            "kc": kc_h.astype(np.float32),
            "wv": wv_h.astype(np.float32),
            "vc": vc_h.astype(np.float32),
            "wp": wp_h.astype(np.float32),
            "relT": relT_h,
        })
    return in_maps


_WP = None


def Wp_slice_T(g):
    # wp rhs chunks: [2, 96, C]; row j of chunk ch = Wp[:, g*GC + ch*96 + j].T
    ds = slice(g * GC, (g + 1) * GC)
    return np.ascontiguousarray(_WP[:, ds].T.astype(np.float32))


def kernel(**inputs):
    global _WP
    x = np.asarray(inputs["x"], np.float32)
    relative_pos = np.asarray(inputs["relative_pos"], np.float32)
    Wq = np.asarray(inputs["Wq"], np.float32)
    Wk = np.asarray(inputs["Wk"], np.float32)
    Wv = np.asarray(inputs["Wv"], np.float32)
    Wp = np.asarray(inputs["Wp"], np.float32)
    bp = np.asarray(inputs["bp"], np.float32)
    sr_w = np.asarray(inputs["sr_w"], np.float32)
    sr_b = np.asarray(inputs["sr_b"], np.float32)
    bn_gamma = np.asarray(inputs["bn_gamma"], np.float32)
    bn_beta = np.asarray(inputs["bn_beta"], np.float32)
    bn_mean = np.asarray(inputs["bn_mean"], np.float32)
    bn_var = np.asarray(inputs["bn_var"], np.float32)
    _WP = Wp

    if "nc" not in _CACHE:
        _CACHE["nc"] = build_program()
    nc = _CACHE["nc"]

    in_maps = _prep_core_inputs(
        x, relative_pos, Wq, Wk, Wv, sr_w, sr_b,
        bn_gamma, bn_beta, bn_mean, bn_var,
    )

    trace = os.environ.get("TRNK_TRACE", "0") == "1"
    res = run_bass_kernel_spmd(nc, in_maps, core_ids=list(range(8)), trace=trace)
    _CACHE["last_results"] = res

    out = np.empty((B, N, C), np.float32)
    for b in range(B):
        out[b] = res.results[2 * b]["out"] + res.results[2 * b + 1]["out"] + bp
    return out
